# revision 6
# baseline (speedup 1.0000x reference)
"""Trainium2 Bass kernel for nn_EquivariantProductBasisBlock.

Computation (per node n, channel c):
  s = nf[n,c,0]; v = nf[n,c,1:4]; v2 = |v|^2
  out0 = w0*s + w1*s^2 + w2*(v2/sqrt3) + w3*s^3 + w4*s*v2        (w_p = W0[sp[n],p,c])
  B1   = u0 + u1*sqrt2*s + u2*sqrt3*s^2 + u3*sqrt(3/5)*v2        (u_p = W1[sp[n],p,c])
  out1m = B1 * v_m
  y0 = out0 @ L0 / sqrt(C);  y1m = out1m @ L1 / sqrt(C)
  y[n,c,:] = [y0, y1x, y1y, y1z] + sc[n,c,:]

Sharding: data-parallel over nodes across 8 cores (8192 nodes/core).

Device mapping per 128-node tile (node-major [node partitions x free]):
  - per-node path weights via one-hot gather matmul on PE (K=10)
  - polynomial features via a fused Horner chain on the Vector engine,
    with W0/W1 path planes packed in paired column order so pairs/triples
    of ops fuse into single wide tensor_tensor instructions
  - v2 via one strided square + grouped tensor_reduce
  - out0/out1 transposed on PE (needed: channel contraction runs over
    the partition dim), channel-mixing matmuls on PE
  - sc added by identity-matmul accumulation into the same PSUM bank
  - one strided ACT copy interleaves [y0|y1x|y1y|y1z] -> [n, c, 4]
"""

import numpy as np

N_CORES = 8
N_NODES = 65536
C = 128
E = 10
NODES_PER_CORE = N_NODES // N_CORES          # 8192
TILES_PER_CORE = NODES_PER_CORE // 128       # 64
TILES_PER_CHUNK = 4                          # 512 nodes -> 1 MiB per DMA
CHUNKS = TILES_PER_CORE // TILES_PER_CHUNK   # 16

INV_SQ3 = 1.0 / np.sqrt(3.0)
SQ2 = float(np.sqrt(2.0))
SQ3 = float(np.sqrt(3.0))
SQ35 = float(np.sqrt(3.0 / 5.0))

_CACHE = {}


# ---------------------------------------------------------------------------
# Workarounds for the walrus build in this container: it rejects any
# instruction carrying more than one sync-wait ("Too many sync wait
# commands").  Split extra waits onto same-engine NOPs preceding the
# instruction (identical semantics: the engine queue is FIFO).
# ---------------------------------------------------------------------------
def _apply_patches():
    import concourse.tile as tile
    from concourse import mybir
    from concourse.vector_clock import ScopedClock

    if getattr(tile.TileContext, "_singlewait_patched", False):
        return

    def _patched_drain_and_barrier(self, tick_clock, wait_clock):
        nc = self.nc
        probe = nc.sync.nop()
        wait_clock.add_sem_waits(probe.ins, ScopedClock({None: tick_clock.global_clock}))
        si = probe.ins.sync_info
        waits = list(si.on_wait) if si and si.on_wait else []
        if len(waits) > 1:
            probe.ins.sync_info = type(si)(on_wait=waits[:1], on_update=[])
            for w in waits[1:]:
                extra = nc.sync.nop()
                extra.ins.sync_info = type(si)(on_wait=[w], on_update=[])
        nc.sync.drain()
        nc.all_engine_barrier()
        assert self.sems is not None
        popped = nc._tile_sem_poison_stack.pop()
        assert popped is self._sem_poison
        nc.clear_and_free_semaphores(list(self.sems.allocated().values()))
        nc.all_engine_barrier()

    _orig_commit = tile.TileContext._commit_instruction

    def _split_commit(self, inst, lazy_reg_writes=True):
        si = getattr(inst, "sync_info", None)
        if (si is not None and si.on_wait and len(si.on_wait) > 1
                and getattr(inst, "engine", mybir.EngineType.Unassigned)
                != mybir.EngineType.Unassigned):
            waits = list(si.on_wait)
            for w in waits[:-1]:
                nop = mybir.InstNoOp(name=self.nc.get_next_instruction_name(),
                                     ins=[], outs=[], engine=inst.engine)
                nop.sync_info = mybir.SyncInfo(on_wait=[w], on_update=[])
                _orig_commit(self, nop, lazy_reg_writes=False)
            inst.sync_info = mybir.SyncInfo(on_wait=[waits[-1]],
                                            on_update=list(si.on_update or []))
        return _orig_commit(self, inst, lazy_reg_writes)

    tile.TileContext._drain_and_barrier = _patched_drain_and_barrier
    tile.TileContext._commit_instruction = _split_commit
    tile.TileContext._singlewait_patched = True


def _build_program(reps=1):
    import concourse.bass as bass
    import concourse.tile as tile
    from concourse import mybir
    from concourse.masks import make_identity

    _apply_patches()
    F32 = mybir.dt.float32
    nc = bass.Bass()

    nf_d = nc.declare_dram_parameter("nf", [NODES_PER_CORE, 512], F32, isOutput=False)
    sc_d = nc.declare_dram_parameter("sc", [NODES_PER_CORE, 512], F32, isOutput=False)
    att_d = nc.declare_dram_parameter("att", [E, NODES_PER_CORE], F32, isOutput=False)
    w_d = nc.declare_dram_parameter("w01", [E, 1152], F32, isOutput=False)
    l0_d = nc.declare_dram_parameter("l0", [C, C], F32, isOutput=False)
    l1_d = nc.declare_dram_parameter("l1", [C, C], F32, isOutput=False)
    out_d = nc.declare_dram_parameter("out", [NODES_PER_CORE, 512], F32, isOutput=True)

    # chunk views: (chunk, part, tile-in-chunk, 512)
    nf_v = nf_d.rearrange("(cc a p) f -> cc p a f", a=TILES_PER_CHUNK, p=128)
    sc_v = sc_d.rearrange("(cc a p) f -> cc p a f", a=TILES_PER_CHUNK, p=128)
    out_v = out_d.rearrange("(cc a p) f -> cc p a f", a=TILES_PER_CHUNK, p=128)

    mult = mybir.AluOpType.mult
    add = mybir.AluOpType.add

    from contextlib import ExitStack

    with tile.TileContext(nc) as tc, ExitStack() as ctx:
        consts = ctx.enter_context(tc.tile_pool(name="consts", bufs=1))
        chunks = ctx.enter_context(tc.tile_pool(name="chunks", bufs=2))
        work = ctx.enter_context(tc.tile_pool(name="work", bufs=2))
        psW = ctx.enter_context(tc.tile_pool(name="psW", bufs=2, space="PSUM"))
        psT = ctx.enter_context(tc.tile_pool(name="psT", bufs=1, space="PSUM"))
        psY = ctx.enter_context(tc.tile_pool(name="psY", bufs=1, space="PSUM"))

        t_w = consts.tile([E, 1152], F32)
        nc.sync.dma_start(out=t_w, in_=w_d[:, :])
        t_l0 = consts.tile([C, C], F32)
        nc.sync.dma_start(out=t_l0, in_=l0_d[:, :])
        t_l1 = consts.tile([C, C], F32)
        nc.sync.dma_start(out=t_l1, in_=l1_d[:, :])
        t_att = consts.tile([E, NODES_PER_CORE], F32)
        nc.sync.dma_start(out=t_att, in_=att_d[:, :])
        ident = consts.tile([C, C], F32)
        make_identity(nc, ident)

        for cc in [c for _ in range(reps) for c in range(CHUNKS)]:
            t_nf = chunks.tile([128, TILES_PER_CHUNK, 512], F32, tag="nf")
            nc.sync.dma_start(out=t_nf, in_=nf_v[cc])
            t_sc = chunks.tile([128, TILES_PER_CHUNK, 512], F32, tag="sc")
            nc.sync.dma_start(out=t_sc, in_=sc_v[cc])
            t_y = chunks.tile([128, TILES_PER_CHUNK, 512], F32, tag="y")

            for a in range(TILES_PER_CHUNK):
                it = cc * TILES_PER_CHUNK + a
                n0 = it * 128
                o_nf = t_nf.offset + a * 512
                o_sc = t_sc.offset + a * 512
                part = t_nf.ap[0]

                def nfap(off, *dims):
                    return bass.AP(tensor=t_nf.tensor, offset=o_nf + off,
                                   ap=[part, *dims])

                s1 = nfap(0, [4, 128])
                s2 = nfap(0, [0, 2], [4, 128])
                s3 = nfap(0, [0, 3], [4, 128])
                vcm = nfap(1, [4, 128], [1, 3])    # (c outer, m inner)
                vmc = nfap(1, [1, 3], [4, 128])    # (m outer, c inner)

                # --- per-node path weights: one-hot gather matmul (K=10) ---
                p_w = psW.tile([128, 1152], F32, tag="pw")
                att_sl = t_att[:, n0:n0 + 128]
                # each matmul is the first writer of its own PSUM bank this
                # iteration -> each needs start=True (start clears has_written
                # only for the bank it writes; stale set bits would turn the
                # write into an accumulate on pool reuse)
                nc.tensor.matmul(p_w[:, 0:512], lhsT=att_sl, rhs=t_w[:, 0:512],
                                 start=True, stop=True)
                nc.tensor.matmul(p_w[:, 512:1024], lhsT=att_sl, rhs=t_w[:, 512:1024],
                                 start=True, stop=True)
                nc.tensor.matmul(p_w[:, 1024:1152], lhsT=att_sl, rhs=t_w[:, 1024:1152],
                                 start=True, stop=True)

                # --- v2 = vx^2 + vy^2 + vz^2 ---
                t_vsq = work.tile([128, 128, 3], F32, tag="vsq")
                nc.vector.tensor_tensor(out=t_vsq, in0=vcm, in1=vcm, op=mult)
                t_v2 = work.tile([128, 128], F32, tag="v2")
                nc.vector.tensor_reduce(out=t_v2, in_=t_vsq,
                                        axis=mybir.AxisListType.X, op=add)

                # --- fused Horner chain ---
                # W column layout: [w3|u2|w4] [w1|u1|w2] [w0|u0] [u3]
                T1 = work.tile([128, 384], F32, tag="t1")
                nc.vector.tensor_tensor(out=T1, in0=s3, in1=p_w[:, 0:384], op=mult)
                nc.vector.tensor_tensor(out=T1, in0=T1, in1=p_w[:, 384:768], op=add)
                nc.vector.tensor_tensor(out=T1[:, 0:256], in0=T1[:, 0:256],
                                        in1=s2, op=mult)
                nc.vector.tensor_tensor(out=T1[:, 0:256], in0=T1[:, 0:256],
                                        in1=p_w[:, 768:1024], op=add)
                # T1 = [h2 | b2 | g]:  h2 = w0 + s*w1 + s^2*w3
                #                      b2 = u0 + s*u1 + s^2*u2,  g = w2 + s*w4
                t_X = work.tile([128, 512], F32, tag="x")      # [out0|o1x|o1y|o1z]
                t_h3 = work.tile([128, 128], F32, tag="h3")
                nc.vector.tensor_tensor(out=t_h3, in0=T1[:, 0:128], in1=s1, op=mult)
                t_gv = work.tile([128, 128], F32, tag="gv")
                nc.vector.tensor_tensor(out=t_gv, in0=T1[:, 256:384], in1=t_v2, op=mult)
                nc.vector.tensor_tensor(out=t_X[:, 0:128], in0=t_h3, in1=t_gv, op=add)
                t_q = work.tile([128, 128], F32, tag="q")
                nc.vector.tensor_tensor(out=t_q, in0=t_v2, in1=p_w[:, 1024:1152], op=mult)
                t_B1 = work.tile([128, 128], F32, tag="b1")
                nc.vector.tensor_tensor(out=t_B1, in0=T1[:, 128:256], in1=t_q, op=add)
                # out1m = B1 * v_m  (one wide op, m-major)
                b1_b3 = bass.AP(tensor=t_B1.tensor, offset=t_B1.offset,
                                ap=[t_B1.ap[0], [0, 3], [1, 128]])
                xo = bass.AP(tensor=t_X.tensor, offset=t_X.offset + 128,
                             ap=[t_X.ap[0], [128, 3], [1, 128]])
                nc.vector.tensor_tensor(out=xo, in0=b1_b3, in1=vmc, op=mult)

                # --- transposes (channel contraction needs c on partitions) ---
                p_T = psT.tile([128, 512], F32, tag="pt")
                for k in range(4):
                    nc.tensor.matmul(p_T[:, k * 128:(k + 1) * 128],
                                     lhsT=t_X[:, k * 128:(k + 1) * 128], rhs=ident,
                                     is_transpose=True,
                                     start=(k == 0), stop=(k == 3))
                t_XT = work.tile([128, 512], F32, tag="xt")
                nc.scalar.copy(out=t_XT, in_=p_T)

                # --- channel-mixing matmuls + sc accumulation ---
                p_Y = psY.tile([128, 512], F32, tag="py")
                for k in range(4):
                    nc.tensor.matmul(p_Y[:, k * 128:(k + 1) * 128],
                                     lhsT=t_XT[:, k * 128:(k + 1) * 128],
                                     rhs=(t_l0 if k == 0 else t_l1),
                                     start=(k == 0), stop=False)
                for k in range(4):
                    sck = bass.AP(tensor=t_sc.tensor, offset=o_sc + k,
                                  ap=[t_sc.ap[0], [4, 128]])
                    nc.tensor.matmul(p_Y[:, k * 128:(k + 1) * 128],
                                     lhsT=ident, rhs=sck,
                                     start=False, stop=(k == 3))

                # --- interleave copy PSUM -> y chunk ---
                il_out = bass.AP(tensor=t_y.tensor, offset=t_y.offset + a * 512,
                                 ap=[t_y.ap[0], [1, 4], [4, 128]])
                il_in = bass.AP(tensor=p_Y.tensor, offset=p_Y.offset,
                                ap=[p_Y.ap[0], [128, 4], [1, 128]])
                nc.scalar.copy(out=il_out, in_=il_in)

            nc.sync.dma_start(out=out_v[cc], in_=t_y)

    return nc


def _prep_host(inputs):
    nf = np.ascontiguousarray(np.asarray(inputs["node_feats"], dtype=np.float32))
    sc = np.ascontiguousarray(np.asarray(inputs["sc"], dtype=np.float32))
    sp = np.asarray(inputs["node_species"])
    W0 = np.asarray(inputs["W0"], dtype=np.float32)
    W1 = np.asarray(inputs["W1"], dtype=np.float32)
    L0 = np.asarray(inputs["L0"], dtype=np.float32)
    L1 = np.asarray(inputs["L1"], dtype=np.float32)

    att = (sp[None, :] == np.arange(E, dtype=sp.dtype)[:, None]).astype(np.float32)

    w0 = W0.copy()
    w0[:, 2, :] *= INV_SQ3
    u = W1.copy()
    u[:, 1, :] *= SQ2
    u[:, 2, :] *= SQ3
    u[:, 3, :] *= SQ35
    # column layout: [w3|u2|w4] [w1|u1|w2] [w0|u0] [u3]
    w01 = np.concatenate([
        w0[:, 3, :], u[:, 2, :], w0[:, 4, :],
        w0[:, 1, :], u[:, 1, :], w0[:, 2, :],
        w0[:, 0, :], u[:, 0, :],
        u[:, 3, :],
    ], axis=1).astype(np.float32)

    inv_sqrt_c = np.float32(1.0 / np.sqrt(C))
    l0 = np.ascontiguousarray(L0 * inv_sqrt_c)
    l1 = np.ascontiguousarray(L1 * inv_sqrt_c)
    return nf, sc, att, w01, l0, l1


def kernel(**inputs):
    from concourse.bass_utils import run_bass_kernel_spmd

    nf, sc, att, w01, l0, l1 = _prep_host(inputs)
    nf2 = nf.reshape(N_NODES, 512)
    sc2 = sc.reshape(N_NODES, 512)

    if "nc" not in _CACHE:
        _CACHE["nc"] = _build_program()
    nc = _CACHE["nc"]

    in_maps = []
    for c in range(N_CORES):
        lo, hi = c * NODES_PER_CORE, (c + 1) * NODES_PER_CORE
        in_maps.append({
            "nf": nf2[lo:hi],
            "sc": sc2[lo:hi],
            "att": np.ascontiguousarray(att[:, lo:hi]),
            "w01": w01,
            "l0": l0,
            "l1": l1,
        })

    res = run_bass_kernel_spmd(nc, in_maps, core_ids=list(range(N_CORES)),
                               **_CACHE.get("run_kwargs", {}))
    _CACHE["last_result"] = res
    y = np.concatenate([res.results[c]["out"] for c in range(N_CORES)], axis=0)
    return y.reshape(N_NODES, C, 4)
